# revision 29
# baseline (speedup 1.0000x reference)
"""Trainium2 Bass kernel for nn_CoAttentionFusionBlock.

Math: the reference's softmax is over a singleton dim, so its weights are
exactly 1.0 and o1/o2 equal the raw features bit-for-bit. The module reduces to

    out = concat([feat_depth, feat_rgb], axis=1) @ W_f.T + b_f        # [B, D]

W_k1/b_k1/W_k2/b_k2 only feed the (dead) score path and are never needed.

Distribution: pure data parallel over the batch dim across 8 NeuronCores.
Each core computes yT = WfT.T @ xT with operands pre-transposed on host so the
contraction dim (2048) lands on SBUF partitions.

v5 design (on top of the v3 bf16 pipeline; each item traced on HW):
  * bf16 operands for 14 of 16 k-tiles; the last 2 k-tiles (K=256) are fp8
    (TRN float8e4 == ml_dtypes.float8_e4m3, max +-240) fed to ONE
    perf_mode=DoubleRow matmul per accumulation group. DoubleRow virtualizes
    the PE to 128x256 (2 fp8 weights/cell, 2 MACs/cycle), replacing 2 bf16
    matmuls (2x216 ns) with one DR matmul whose 256-col LDWEIGHTS (~213 ns,
    no FWL) hides under the previous bf16 fill and whose own fill is ~120 ns.
    Scales sx=0.5 / sw=2.0 multiply to 1 so PSUM needs no rescale.
    Accuracy (measured, deterministic for the fixed seed): 1.55e-2 vs the
    fp32 reference (gate 2e-2); bf16-only path is 2.3e-3.
  * PE warmup: ~26 dummy matmuls (M=32, N=256, memset operand) issue as the
    first Tensor instructions, running 5->9.5 us while the first DMAs land.
    They flip the HAM clock gate to K=8/8 (~3.4 us busy window) so the real
    matmul stream starts at 2.4 GHz instead of paying ~2.4 us of 1.2 GHz
    cold ramp (v3 trace: 11 cold matmuls at 427 ns cadence).
  * Head: first loads reordered w[t0]; x0[t0]; x0[t1]; (w[t], x0[t+1])...;
    bias last (bias isn't needed until the first evict at ~40 us; in v3 its
    dma_start sat in front of everything, costing ~0.7 us of the ~0.61 us/
    dma_start single-queue issue rate before the critical w[t0]).
  * Stores are bf16 (yT dram param is bf16, host upcasts): halves store
    bytes on the shared in-order HWDGE queue and the DVE evict gets the
    16-bit 2x path. Output quantization adds ~4e-4 max-rel.
  * Tail: the final (s=7, j=7) group is split into 256/128/128-col
    accumulation groups so the last group's evict+store moves 32 KB instead
    of 256 KB; earlier split-groups' evicts hide under the later groups'
    matmuls (v3 trace: 7.0 us from last matmul to exec end).
  * x is packed ON HOST into SBUF layout (16 KB contiguous per partition,
    slab) -- naive [K, B] layout had 1 KB lines and saturated the HWDGE
    queue at ~133 GB/s, starving the PE (v2 lesson).
  * All loads on the sync HWDGE queue, all issued before any store
    dma_start except slabs 5-7, which issue from inside the compute stream
    (after slab s evictions) to avoid head-of-line-blocking the o_sb pool
    (v3 lesson: 8.5 us PE stall otherwise).

Measured (8-core SPMD, neuron-profile): 224.2-226.5 us (run-to-run launch
jitter ~0.5 us, occasional ~+3 us outlier from a late DMA ramp) vs 242.7 us
for the v3 all-bf16 kernel. Budget: ~7.2 us fixed engine prologue, ~3.3 us
DMA ramp to first matmul, ~208 us gapless PE stream (64 groups x
(14x216 + 228) ns; the all-bf16 floor is 221.2 us), ~4.5 us tail
(evict + 32 KB store + fixed ~2.2 us end barrier). Six ~227 ns PE hiccups
recur at exactly 32525 ns period across all runs -- external housekeeping,
not addressable. Head bandwidth is chip-HBM-capped (8 cores x ~350 GB/s),
so a second DMA queue cannot help; load ORDER is what matters and is
arranged just-in-time above.
"""

import numpy as np
import ml_dtypes

import concourse.bacc as bacc
import concourse.mybir as mybir
import concourse.tile as tile
from concourse.bass_utils import run_bass_kernel_spmd

B = 32768
D = 1024
NCORES = 8
BLOC = B // NCORES  # 4096 batch rows per core
K = 2 * D  # 2048 contraction dim
P = 128  # partitions
NT = 512  # moving free dim per matmul (one PSUM bank of fp32)
KT = K // P  # 16 k-tiles total
JT = D // P  # 8 output-row tiles
ST = BLOC // NT  # 8 slabs of 512 batch cols

USE_DR = True  # last 2 k-tiles via one fp8 DoubleRow matmul per group
KB = KT - 2 if USE_DR else KT  # bf16 k-tiles
SLAB_ELEMS = KB * NT  # bf16 elems per partition per slab
SX = 0.5  # fp8 x scale (sx*sw == 1 -> no PSUM rescale)
SW = 2.0  # fp8 w scale
N_WARM = 12  # dummy warmup matmuls (M=32, N=256); sized so the dummies end
# right as the first w/x tiles land (~10.3 us) -- the HAM flip needs ~3.4 us
# of continuous PE busy, which dummies+first real matmuls together provide
# (v5 trace: 26 dummies ran to 12.2 us while data was ready at ~10.3).

FP32 = mybir.dt.float32
BF16 = mybir.dt.bfloat16
FP8 = mybir.dt.float8e4
NP_BF16 = np.dtype(ml_dtypes.bfloat16)
NP_FP8 = np.dtype(ml_dtypes.float8_e4m3)  # bit-identical to TRN float8e4

# test.py can flip these to profile; harness leaves them alone.
TRACE = False
TRACE_DIR = None
LAST_RESULT = None
DT_IN = "bf16+fp8dr" if USE_DR else "bf16"


def _build_nc():
    # Bacc (not raw Bass): its compile() runs move_matmul_waits_to_ldweights +
    # generate_event_semaphores, which split sync waits to <=1 per instruction
    # (TRN2 HW limit -- raw Bass hits "Too many sync wait commands" in walrus).
    nc = bacc.Bacc(None)
    # xP is host-packed to SBUF order: xP[p, ((s*KB + t)*NT + b)]
    #   = concat(feat_depth, feat_rgb).T[t*P + p, s*NT + b]
    xP = nc.declare_dram_parameter("xP", [P, ST * KB * NT], BF16, isOutput=False)
    # wP is host-packed to SBUF order [p, t*D + j] so any t-range loads as
    # ONE 128-line dma_start (issue cost ~0.61 us scales with line count,
    # not bytes -- a [t, p, j]-layout pair would be 256 lines and save nil)
    wP = nc.declare_dram_parameter("wP", [P, KB * D], BF16, isOutput=False)
    biasT = nc.declare_dram_parameter("biasT", [P, JT], FP32, isOutput=False)
    yT = nc.declare_dram_parameter("yT", [D, BLOC], BF16, isOutput=True)
    if USE_DR:
        # pair dim i of DoubleRow: logical k = KB*128 + i*128 + p
        x8P = nc.declare_dram_parameter("x8P", [P, ST, 2, NT], FP8, isOutput=False)
        w8P = nc.declare_dram_parameter("w8P", [P, 2, D], FP8, isOutput=False)

    yT_v = yT.rearrange("(j p) b -> j p b", p=P)  # [JT, 128, BLOC]

    with tile.TileContext(nc) as tc:
        with (
            tc.tile_pool(name="wpool", bufs=1) as wpool,
            tc.tile_pool(name="xpool", bufs=1) as xpool,
            tc.tile_pool(name="opool", bufs=6) as opool,
            tc.tile_pool(name="bpool", bufs=1) as bpool,
            tc.tile_pool(name="psum", bufs=8, space="PSUM") as psum_pool,
        ):
            # --- PE warmup: run dummy matmuls on memset data while the first
            # loads are in flight, so the HAM clock gate is at K=8/8 (2.4 GHz)
            # when the real stream starts. M=32 keeps LDWEIGHTS at ~27 ns.
            dummy_sb = bpool.tile([P, 256], BF16)
            nc.vector.memset(dummy_sb[:], 0.0)
            ps_dummy = psum_pool.tile([P, NT], FP32, tag="ps", name="ps_dummy")
            for _ in range(N_WARM):
                nc.tensor.matmul(
                    ps_dummy[0:32, 0:256],
                    dummy_sb[:, 0:32],
                    dummy_sb[:, 0:256],
                    start=True,
                    stop=True,
                )

            bias_sb = bpool.tile([P, JT], FP32)
            w_sb = wpool.tile([P, KB * D], BF16)
            x_sb = [
                xpool.tile([P, SLAB_ELEMS], BF16, tag=f"x{s}", name=f"x_sb{s}")
                for s in range(ST)
            ]
            if USE_DR:
                x8_sb = xpool.tile([P, ST, 2, NT], FP8, tag="x8", name="x8_sb")
                w8_sb = wpool.tile([P, 2, D], FP8, tag="w8", name="w8_sb")

            def load_w(t0, t1):
                nc.sync.dma_start(
                    out=w_sb[:, t0 * D : t1 * D], in_=wP[:, t0 * D : t1 * D]
                )

            def load_x0(t0, t1):
                nc.sync.dma_start(
                    out=x_sb[0][:, t0 * NT : t1 * NT],
                    in_=xP[:, t0 * NT : t1 * NT],
                )

            def load_slab(eng, s):
                eng.dma_start(
                    out=x_sb[s][:],
                    in_=xP[:, s * SLAB_ELEMS : (s + 1) * SLAB_ELEMS],
                )

            # Startup stream on the single sync HWDGE queue (~0.61 us issue
            # per dma_start): exactly the order slab 0's t-outer loop consumes,
            # bias deferred (first needed at the ~40 us first evict).
            # The head is ISSUE-RATE bound (~0.61 us per dma_start on the
            # single sync queue) on top of ~250 GB/s effective HBM under
            # 8-core contention -- v6 trace: slab 1 landed at 39.8 us vs its
            # 37.9 us deadline, stalling the PE 2.1 us. So: pair up the x0
            # loads (8 starts, not 14), load only slab 0's 128 KB slice of
            # x8 before slab 1, and defer the rest.
            # NOTE (v13 lesson): dma_start "issue time" is HWDGE-ring
            # backpressure paced by transfer drain, so coalescing w rows
            # into pairs saves nothing and the coarser completion semaphore
            # made the t=1 weight deadline ~1 us later (2.4 us PE stall).
            # Keep single w rows: finest-grained just-in-time completions.
            # x0 as SINGLES too (not pairs): descriptor-gen runs ahead of
            # transfer drain at the head, so finer dma_starts are free and
            # each odd tile's completion advances ~0.37 us -- degraded-run
            # traces showed 640-695 ns MM waits gated on the PAIR semaphores
            # every other t-row.
            load_w(0, 1)
            load_x0(0, 1)
            for t in range(1, KB):
                load_w(t, t + 1)
                load_x0(t, t + 1)
                if t == 6:
                    # DR operands mid-stream: needed by slab 0's DR at ~35 us,
                    # land ~20-25 us. Not at the very head (crowds the
                    # just-in-time w/x ramp), not after slab 1 (v7 trace:
                    # x8[s0] landed 38.8 us and stalled the DR section 2 us).
                    # bias (4 KB, needed by the FIRST evict ~38 us) rides
                    # here too -- parked after the bulk loads it landed ~44
                    # in degraded runs and stalled the evict chain.
                    if USE_DR:
                        nc.sync.dma_start(out=w8_sb[:, :, :], in_=w8P[:, :, :])
                        nc.sync.dma_start(
                            out=x8_sb[:, 0, :, :], in_=x8P[:, 0, :, :]
                        )
                    nc.sync.dma_start(out=bias_sb[:], in_=biasT[:, :])
            # Slab 1 in HALVES by deadline: its first group consumes tiles
            # t0..t13 sequentially over 3.25 us, so the t0-6 half gates the
            # group start (~36.3 us) while t7-13 isn't needed until ~37.8.
            # Finer completion granularity moves the binding cumulative-
            # bytes deadline ~0.9 MB (~2.5 us) earlier -- mid-run traces
            # stalled 2-3.3 us in exactly this 36-42 us window. x8[s1]
            # (needed ~39.5 us by slab 1's first DR) likewise gets its own
            # 128 KB transfer ahead of the bulk x8 load.
            # Remaining loads strictly by consumption deadline: slab1 halves
            # (36.3/37.8 us), x8[s1] (39.5), slab2 halves (62.3/63.8),
            # x8[s2] (65.6), slab3 (88.3), x8[s3:] (91.6+), slab4 (114.3).
            half = 7 * NT
            for s in (1, 2):
                nc.sync.dma_start(
                    out=x_sb[s][:, 0:half],
                    in_=xP[:, s * SLAB_ELEMS : s * SLAB_ELEMS + half],
                )
                nc.sync.dma_start(
                    out=x_sb[s][:, half:SLAB_ELEMS],
                    in_=xP[:, s * SLAB_ELEMS + half : (s + 1) * SLAB_ELEMS],
                )
                if USE_DR:
                    nc.sync.dma_start(
                        out=x8_sb[:, s, :, :], in_=x8P[:, s, :, :]
                    )
            load_slab(nc.sync, 3)
            if USE_DR:
                nc.sync.dma_start(out=x8_sb[:, 3:, :, :], in_=x8P[:, 3:, :, :])
            load_slab(nc.sync, 4)

            def mm(ps, j, t, s, start, c0=0, c1=NT):
                nc.tensor.matmul(
                    ps[:, 0 : c1 - c0],
                    w_sb[:, t * D + j * P : t * D + (j + 1) * P],
                    x_sb[s][:, t * NT + c0 : t * NT + c1],
                    start=start,
                    stop=(not USE_DR) and t == KB - 1,
                )

            def mm_dr(ps, j, s, c0=0, c1=NT):
                nc.tensor.matmul(
                    ps[:, 0 : c1 - c0],
                    w8_sb[:, :, j * P : (j + 1) * P],
                    x8_sb[:, s, :, c0:c1],
                    start=False,
                    stop=True,
                    perf_mode=mybir.MatmulPerfMode.DoubleRow,
                )

            def evict(ps, j, s, c0=0, c1=NT, chunks=1):
                o_sb = opool.tile([P, NT], BF16, tag="o", name="o_sb")
                cw = (c1 - c0) // chunks
                for c in range(chunks):
                    sl = slice(c * cw, (c + 1) * cw)
                    nc.vector.tensor_scalar_add(
                        o_sb[:, sl], ps[:, sl], bias_sb[:, j : j + 1]
                    )
                    nc.sync.dma_start(
                        out=yT_v[
                            j, :, s * NT + c0 + c * cw : s * NT + c0 + (c + 1) * cw
                        ],
                        in_=o_sb[:, sl],
                    )

            # Slab 0: t-outer with all 8 PSUM groups open -- each weight
            # k-tile unlocks 8 matmuls, PE streams during the weight load.
            ps0 = [psum_pool.tile([P, NT], FP32, tag="ps", name="ps") for _ in range(JT)]
            for t in range(KB):
                for j in range(JT):
                    mm(ps0[j], j, t, 0, start=(t == 0))
            for j in range(JT):
                if USE_DR:
                    mm_dr(ps0[j], j, 0)
                evict(ps0[j], j, 0)

            # Slabs 1-7: j-outer, one group per accumulation chain, rotating
            # PSUM banks. Late slab loads interleave after slab s evictions.
            for s in range(1, ST):
                if s <= 3:
                    load_slab(nc.sync, s + 4)
                for j in range(JT):
                    last = s == ST - 1 and j == JT - 1
                    if not last:
                        ps = psum_pool.tile([P, NT], FP32, tag="ps", name="ps")
                        for t in range(KB):
                            mm(ps, j, t, s, start=(t == 0))
                        if USE_DR:
                            mm_dr(ps, j, s)
                        evict(ps, j, s)
                        continue
                    # Final group split 256/128/128: earlier splits' evicts
                    # hide under later splits' matmuls and all three land in
                    # ONE shared o_sb tile. Cols 0:384 store right after
                    # evict-b (issue+packets hide under the last group's
                    # matmuls); only a 32 KB store remains after the very
                    # last evict (v10 trace: a single merged 512-col store
                    # paid ~1.2 us of cold-queue packet drain post-matmul).
                    o_tail = opool.tile([P, NT], BF16, tag="o", name="o_tail")
                    for c0, c1 in [(0, 256), (256, 384), (384, 512)]:
                        ps = psum_pool.tile([P, c1 - c0], FP32, tag="ps", name="ps")
                        for t in range(KB):
                            mm(ps, j, t, s, start=(t == 0), c0=c0, c1=c1)
                        if USE_DR:
                            mm_dr(ps, j, s, c0=c0, c1=c1)
                        nc.vector.tensor_scalar_add(
                            o_tail[:, c0:c1], ps[:, 0 : c1 - c0], bias_sb[:, j : j + 1]
                        )
                        if c1 == 384:
                            nc.sync.dma_start(
                                out=yT_v[j, :, s * NT : s * NT + 384],
                                in_=o_tail[:, 0:384],
                            )
                    nc.sync.dma_start(
                        out=yT_v[j, :, s * NT + 384 : (s + 1) * NT],
                        in_=o_tail[:, 384:512],
                    )
    nc.finalize()
    return nc


def kernel(feat_rgb, feat_depth, W_k1, b_k1, W_k2, b_k2, W_f, b_f):
    global LAST_RESULT
    feat_rgb = np.asarray(feat_rgb, dtype=np.float32)
    feat_depth = np.asarray(feat_depth, dtype=np.float32)
    W_f = np.asarray(W_f, dtype=np.float32)
    b_f = np.asarray(b_f, dtype=np.float32)

    WfT = np.ascontiguousarray(W_f.T)  # [2048, 1024] fp32
    # pack bf16 weights to SBUF order [p, t, j] (see wP declaration)
    wP_bf = np.ascontiguousarray(
        WfT[: KB * P].astype(NP_BF16).reshape(KB, P, D).transpose(1, 0, 2)
    ).reshape(P, KB * D)
    biasT = np.ascontiguousarray(b_f.reshape(JT, P).T)  # [128, 8]
    if USE_DR:
        # w8P[p, i, m] = WfT[KB*128 + i*128 + p, m] * SW
        w8P = np.ascontiguousarray(
            (WfT[KB * P :] * SW).reshape(2, P, D).transpose(1, 0, 2)
        ).astype(NP_FP8)
    xd = feat_depth.astype(NP_BF16)
    xr = feat_rgb.astype(NP_BF16)

    in_maps = []
    for i in range(NCORES):
        lo, hi = i * BLOC, (i + 1) * BLOC
        x_cat_T = np.empty((K, BLOC), dtype=NP_BF16)
        x_cat_T[:D] = xd[lo:hi].T
        x_cat_T[D:] = xr[lo:hi].T
        # pack to SBUF order [p, s, t, b]: 14 KB contiguous per (p, slab)
        xPk = np.ascontiguousarray(
            x_cat_T[: KB * P]
            .reshape(KB, P, ST, NT)
            .transpose(1, 2, 0, 3)
            .reshape(P, -1)
        )
        im = {"xP": xPk, "wP": wP_bf, "biasT": biasT}
        if USE_DR:
            # x8P[p, s, i, b] = x_cat_T[KB*128 + i*128 + p, s*NT + b] * SX
            # (quantize from fp32, not the bf16 x_cat_T, to avoid double
            # rounding; slice the original feature block)
            xf32 = np.empty((2 * P, BLOC), dtype=np.float32)
            xf32[:P] = feat_rgb[lo:hi, KB * P - D : KB * P - D + P].T
            xf32[P:] = feat_rgb[lo:hi, KB * P - D + P : KB * P - D + 2 * P].T
            im["x8P"] = np.ascontiguousarray(
                (xf32 * SX)
                .reshape(2, P, ST, NT)
                .transpose(1, 2, 0, 3)
            ).astype(NP_FP8)
            im["w8P"] = w8P
        in_maps.append(im)

    nc = _build_nc()
    res = run_bass_kernel_spmd(
        nc, in_maps, list(range(NCORES)), trace=TRACE, tmpdir=TRACE_DIR
    )
    LAST_RESULT = res

    out = np.empty((B, D), dtype=np.float32)
    for i in range(NCORES):
        out[i * BLOC : (i + 1) * BLOC] = res.results[i]["yT"].astype(np.float32).T
    return out
